# revision 19
# baseline (speedup 1.0000x reference)
"""Multi-head causal attention (B=8,S=1024,D=768,H=12,Dh=64) on 8 TRN2 NeuronCores.

Data-parallel over batch: each core handles one batch element end-to-end
(QKV projection -> causal softmax attention -> output projection). No
collectives. All matmuls run in bf16 (fp32 PSUM accumulation); inputs are
cast/packed to bf16 on the host.

Schedule (vs original baseline):
  - DMA staging: x split in s-halves + per-pair weight chunks, issued across
    the two HW DGE queues (sync/scalar) in consumption-priority order, so the
    PE starts at ~10.5us instead of stalling to ~30us. Zero-filled biases are
    never loaded (spec fills them with zeros).
  - PE warmup burst while DMAs stream (HAM clock-gate releases at ~3.4us of
    sustained activity; real matmuls then run at 2.4GHz from the start).
  - qh-major attention sweep with fillers distributed so the PE never idles:
    qh0 sweep carries QK(p2..p5) + V(kt4..7), qh1 sweep carries the output
    projection for s<512 (i3 split across two groups); the final pair's qh1
    is split in two 256-wide chunks so outproj(i4,i5) overlaps its second
    half, and the last stores ship column-split (short tail).
  - Diagonal-block causal masks merged: one DVE multiply per block over both
    heads ([128,2,128] against a doubled triangular constant).
  - PSUM->SBUF copies spread: QK copies on ACT (idle during the qh0 sweep),
    V/z/out on DVE. No bias loads/adds (spec fills biases with zeros).

Per-core layout:
  xA/xB[dt] [128(d), 512(s)] bf16   x^T halves, 6 d-tiles
  qT/kT[p]  [128(he-pair), 1024(s)] bf16  head-pair packed: partitions 0-63 =
            head 2p, partitions 64-127 = head 2p+1 (e fastest)
  v_sb[kt]  [128(s), 768(h e)] bf16  V in natural layout per key-tile
  Scores are kept transposed (S^T[k, q]) so the softmax reduction over k is
  a ones-matmul on PE; exp needs no max-subtraction (|s/8| < ~4 here).
  zT[p]     [128(he-pair), 1024(s)] bf16  normalized attention output
"""
import sys

sys.path.insert(0, "/opt/trn_rl_repo")

import numpy as np

import concourse.bacc as bacc
import concourse.mybir as mybir
from concourse import tile
from concourse import bass_utils
from concourse.bass_interp import get_hw_module

from concourse.masks import make_upper_triangular

F32 = mybir.dt.float32
BF16 = mybir.dt.bfloat16
EXP = mybir.ActivationFunctionType.Exp

B, S, D, H, Dh = 8, 1024, 768, 12, 64
NP = 128          # partitions
DT = D // NP      # 6 d-tiles
ST = S // NP      # 8 s-tiles
KT = S // NP      # 8 k-tiles
NPAIR = H // 2    # 6 head pairs
SCALE = 1.0 / 8.0  # 1/sqrt(Dh)


def _build():
    nc = bacc.Bacc(
        "TRN2",
        target_bir_lowering=False,
        debug=False,
        enable_asserts=False,
        num_devices=8,
    )
    xa_d = nc.dram_tensor("xa", (DT, NP, 512), BF16, kind="ExternalInput")
    xb_d = nc.dram_tensor("xb", (DT, NP, 512), BF16, kind="ExternalInput")
    wq_d = nc.dram_tensor("wq", (NPAIR, NP, DT, NP), BF16, kind="ExternalInput")
    wk_d = nc.dram_tensor("wk", (NPAIR, NP, DT, NP), BF16, kind="ExternalInput")
    wv_d = nc.dram_tensor("wv", (DT, NP, H * Dh), BF16, kind="ExternalInput")
    wo_d = nc.dram_tensor("wo", (NPAIR, NP, D), BF16, kind="ExternalInput")
    out_d = nc.dram_tensor("out", (S, D), F32, kind="ExternalOutput")

    with tile.TileContext(nc) as tc:
        _body(tc, xa_d, xb_d, wq_d, wk_d, wv_d, wo_d, out_d)

    nc.compile()
    return nc


def _body(tc, xa_d, xb_d, wq_d, wk_d, wv_d, wo_d, out_d):
    nc = tc.nc

    with (
        tc.tile_pool(name="const", bufs=1) as const_pool,
        tc.tile_pool(name="qkT", bufs=1) as qkT_pool,
        tc.tile_pool(name="vsb", bufs=1) as v_pool,
        tc.tile_pool(name="zT", bufs=1) as zT_pool,
        tc.tile_pool(name="wsb", bufs=1) as w_pool,
        tc.tile_pool(name="xsb", bufs=1) as x_pool,
        tc.tile_pool(name="pt", bufs=12) as pt_pool,
        tc.tile_pool(name="rcp", bufs=2) as r_pool,
        tc.tile_pool(name="osb", bufs=3) as o_pool,
    ):
        # ---- constants (gpsimd; warm first so PE warmup starts ASAP) ----
        warm = const_pool.tile([NP, 512], BF16, tag="warm")
        nc.gpsimd.memset(warm[:], 0.125)
        ones64 = const_pool.tile([NP, 64], BF16, tag="ones64")
        nc.gpsimd.memset(ones64[:], 1.0)
        tri2 = const_pool.tile([NP, 2, NP], BF16, tag="tri2")
        make_upper_triangular(nc, tri2[:, 0, :], val=1.0, diag=True)
        make_upper_triangular(nc, tri2[:, 1, :], val=1.0, diag=True)

        # ---- persistent tiles ----
        qT = [qkT_pool.tile([NP, S], BF16, tag=f"qT{p}", name=f"qT{p}") for p in range(NPAIR)]
        kT = [qkT_pool.tile([NP, S], BF16, tag=f"kT{p}", name=f"kT{p}") for p in range(NPAIR)]
        v_sb = [v_pool.tile([NP, H * Dh], BF16, tag=f"v{k}", name=f"v{k}") for k in range(KT)]
        zT = [zT_pool.tile([NP, S], BF16, tag=f"zT{p}", name=f"zT{p}") for p in range(NPAIR)]
        xA = [x_pool.tile([NP, 512], BF16, tag=f"xA{dt}", name=f"xA{dt}") for dt in range(DT)]
        xB = [x_pool.tile([NP, 512], BF16, tag=f"xB{dt}", name=f"xB{dt}") for dt in range(DT)]
        wk_sb = [w_pool.tile([NP, DT, NP], BF16, tag=f"wk{p}", name=f"wk{p}") for p in range(NPAIR)]
        wq_sb = [w_pool.tile([NP, DT, NP], BF16, tag=f"wq{p}", name=f"wq{p}") for p in range(NPAIR)]
        wv_sb = w_pool.tile([NP, DT, H * Dh], BF16, tag="wv", name="wv_sb")
        wo_sb = w_pool.tile([NP, NPAIR, D], BF16, tag="wo", name="wo_sb")

        # ---- loads ----
        # x + first-pair weights on the two HW DGE queues (sync/scalar); all
        # later weights on GpSimd's SW DGE so semaphore-reuse waits on the
        # dma_start instructions never block an engine with compute queued
        # behind them (ACT does the QK copies, Sync stores the output).
        nc.sync.dma_start(wk_sb[0][:], wk_d.ap()[0])
        nc.sync.dma_start(wq_sb[0][:], wq_d.ap()[0])
        nc.sync.dma_start(xA[0][:], xa_d.ap()[0])
        nc.sync.dma_start(xA[2][:], xa_d.ap()[2])
        nc.sync.dma_start(xA[4][:], xa_d.ap()[4])
        nc.sync.dma_start(xB[0][:], xb_d.ap()[0])
        nc.sync.dma_start(xB[2][:], xb_d.ap()[2])
        nc.sync.dma_start(xB[4][:], xb_d.ap()[4])

        nc.scalar.dma_start(xA[1][:], xa_d.ap()[1])
        nc.scalar.dma_start(xA[3][:], xa_d.ap()[3])
        nc.scalar.dma_start(xA[5][:], xa_d.ap()[5])
        nc.scalar.dma_start(xB[1][:], xb_d.ap()[1])
        nc.scalar.dma_start(xB[3][:], xb_d.ap()[3])
        nc.scalar.dma_start(xB[5][:], xb_d.ap()[5])

        nc.gpsimd.dma_start(wk_sb[1][:], wk_d.ap()[1])
        nc.gpsimd.dma_start(wq_sb[1][:], wq_d.ap()[1])
        nc.gpsimd.dma_start(wk_sb[2][:], wk_d.ap()[2])
        nc.gpsimd.dma_start(wq_sb[2][:], wq_d.ap()[2])
        nc.gpsimd.dma_start(wk_sb[3][:], wk_d.ap()[3])
        nc.gpsimd.dma_start(wq_sb[3][:], wq_d.ap()[3])
        nc.gpsimd.dma_start(wv_sb[:], wv_d.ap().rearrange("dt p c -> p dt c"))
        nc.gpsimd.dma_start(wk_sb[4][:], wk_d.ap()[4])
        nc.gpsimd.dma_start(wq_sb[4][:], wq_d.ap()[4])
        nc.gpsimd.dma_start(wk_sb[5][:], wk_d.ap()[5])
        nc.gpsimd.dma_start(wq_sb[5][:], wq_d.ap()[5])
        nc.gpsimd.dma_start(wo_sb[:], wo_d.ap().rearrange("q p c -> p q c"))

        with (
            tc.tile_pool(name="psQK", bufs=2, space="PSUM") as psQK,
            tc.tile_pool(name="psS", bufs=2, space="PSUM") as psS,
            tc.tile_pool(name="psZ", bufs=1, space="PSUM") as psZ,
            tc.tile_pool(name="psL", bufs=1, space="PSUM") as psL,
        ):
            # ---- PE warmup: release the HAM clock gate while DMAs stream ----
            for i in range(9):
                wps = psQK.tile([NP, 512], F32, tag="qk", name=f"warm{i}")
                nc.tensor.matmul(wps[:], warm[:, 0:128], warm[:], start=True, stop=True)

            def emit_qk_sc(p, which, sc):
                """One s-half of the K or Q projection for pair p."""
                w_sb, dstT = (wk_sb, kT) if which == 0 else (wq_sb, qT)
                x_half = xA if sc == 0 else xB
                ps = psQK.tile([NP, 512], F32, tag="qk", name=f"qk{p}_{which}_{sc}")
                for dt in range(DT):
                    nc.tensor.matmul(
                        ps[:], w_sb[p][:, dt, :], x_half[dt][:],
                        start=(dt == 0), stop=(dt == DT - 1),
                    )
                nc.scalar.copy(dstT[p][:, sc * 512:(sc + 1) * 512], ps[:])

            def emit_qk(p):
                emit_qk_sc(p, 0, 0)
                emit_qk_sc(p, 1, 0)
                emit_qk_sc(p, 0, 1)
                emit_qk_sc(p, 1, 1)

            def emit_v(kt):
                """V projection for one key tile (all 12 heads)."""
                ps = psS.tile([NP, 1024], F32, tag="st", name=f"vps{kt}")
                for dt in range(DT):
                    lhs = (xA[dt][:, kt * NP:(kt + 1) * NP] if kt < 4
                           else xB[dt][:, (kt - 4) * NP:(kt - 3) * NP])
                    nc.tensor.matmul(ps[:, 0:512], lhs, wv_sb[:, dt, 0:512],
                                     start=(dt == 0), stop=(dt == DT - 1))
                    nc.tensor.matmul(ps[:, 512:768], lhs, wv_sb[:, dt, 512:768],
                                     start=(dt == 0), stop=(dt == DT - 1))
                nc.vector.tensor_copy(v_sb[kt][:], ps[:, 0:768])

            def emit_outproj(i, split_store=False):
                ps1 = psQK.tile([NP, 512], F32, tag="qk", name=f"op{i}a")
                ps2 = psQK.tile([NP, 256], F32, tag="qk", name=f"op{i}b")
                for p2 in range(NPAIR):
                    lhs = zT[p2][:, i * NP:(i + 1) * NP]
                    nc.tensor.matmul(ps1[:], lhs, wo_sb[:, p2, 0:512],
                                     start=(p2 == 0), stop=(p2 == NPAIR - 1))
                for p2 in range(NPAIR):
                    lhs = zT[p2][:, i * NP:(i + 1) * NP]
                    nc.tensor.matmul(ps2[:], lhs, wo_sb[:, p2, 512:768],
                                     start=(p2 == 0), stop=(p2 == NPAIR - 1))
                o_t = o_pool.tile([NP, D], F32, tag="o", name=f"ot{i}")
                nc.vector.tensor_copy(o_t[:, 0:512], ps1[:])
                if split_store:
                    # tail tiles: ship the first 2/3 while ps2 still accumulates
                    nc.sync.dma_start(out_d.ap()[i * NP:(i + 1) * NP, 0:512], o_t[:, 0:512])
                    nc.vector.tensor_copy(o_t[:, 512:768], ps2[:])
                    nc.sync.dma_start(out_d.ap()[i * NP:(i + 1) * NP, 512:768], o_t[:, 512:768])
                else:
                    nc.vector.tensor_copy(o_t[:, 512:768], ps2[:])
                    nc.sync.dma_start(out_d.ap()[i * NP:(i + 1) * NP, :], o_t[:])

            def blocks(qlo, qhi):
                """(kt, c0, w) for every score block intersecting q in [qlo,qhi)."""
                out = []
                for kt in range(KT):
                    c0 = max(kt * NP, qlo)
                    if c0 < qhi:
                        out.append((kt, c0, qhi - c0))
                return out

            def pass1(p, qlo, qhi):
                """Scores + exp (+ causal mask on diagonal blocks)."""
                pts = {}
                for kt, c0, w in blocks(qlo, qhi):
                    q0 = kt * NP
                    st = psS.tile([NP, 2, 512], F32, tag="st", name=f"st{p}_{qlo}_{kt}")
                    for h in range(2):
                        nc.tensor.matmul(
                            st[:, h, 0:w],
                            kT[p][h * 64:(h + 1) * 64, q0:q0 + NP],
                            qT[p][h * 64:(h + 1) * 64, c0:c0 + w],
                            start=True, stop=True,
                        )
                    pt = pt_pool.tile([NP, 2, 512], BF16, tag="pt", name=f"pt{p}_{qlo}_{kt}")
                    nc.scalar.activation(pt[:, :, 0:w], st[:, :, 0:w], EXP, scale=SCALE)
                    if c0 == q0:  # diagonal block: zero out k > q (both heads at once)
                        nc.vector.tensor_mul(pt[:, :, 0:NP], pt[:, :, 0:NP], tri2[:])
                    pts[kt] = (pt, c0, w)
                return pts

            def pass2(p, qlo, qhi, pts):
                """l and z accumulation + softmax normalize into zT."""
                width = qhi - qlo
                bl = blocks(qlo, qhi)
                z_ps = psZ.tile([NP, width], F32, tag="z", name=f"z{p}_{qlo}")
                l_ps = psL.tile([NP, width], F32, tag="l", name=f"l{p}_{qlo}")
                for idx, (kt, c0, w) in enumerate(bl):
                    pt, _, _ = pts[kt]
                    first = idx == 0
                    last = idx == len(bl) - 1
                    o0 = c0 - qlo

                    # pair l(h) with z(1-h): disjoint PE col groups + PSUM banks
                    def mm_l(h):
                        nc.tensor.matmul(
                            l_ps[h * 64:(h + 1) * 64, o0:o0 + w],
                            ones64[:, 0:64], pt[:, h, 0:w],
                            start=first, stop=last, skip_group_check=True,
                        )

                    def mm_z(h):
                        nc.tensor.matmul(
                            z_ps[h * 64:(h + 1) * 64, o0:o0 + w],
                            v_sb[kt][:, (2 * p + h) * 64:(2 * p + h + 1) * 64],
                            pt[:, h, 0:w],
                            start=first, stop=last, skip_group_check=True,
                        )

                    mm_l(0); mm_z(1); mm_l(1); mm_z(0)
                recip = r_pool.tile([NP, 512], F32, tag="rcp", name=f"rcp{p}_{qlo}")
                nc.vector.reciprocal_approx_fast(out=recip[:, 0:width], in_=l_ps[:])
                nc.vector.tensor_mul(zT[p][:, qlo:qhi], z_ps[:], recip[:, 0:width])

            # ---- projections head start ----
            emit_qk(0)
            emit_qk(1)
            for kt in range(4):
                emit_v(kt)

            # ---- qh0 sweep (q 0:512, key tiles 0..3) with fillers ----
            qh0_fill = {0: lambda: emit_qk(2), 1: lambda: emit_qk(3),
                        2: lambda: emit_qk(4), 3: lambda: emit_qk(5),
                        4: lambda: (emit_v(4), emit_v(5)),
                        5: lambda: (emit_v(6), emit_v(7))}
            for p in range(NPAIR):
                pts = pass1(p, 0, 512)
                qh0_fill[p]()
                pass2(p, 0, 512, pts)

            def emit_outproj_a(i):
                ps1 = psQK.tile([NP, 512], F32, tag="qk", name=f"op{i}a")
                for p2 in range(NPAIR):
                    nc.tensor.matmul(ps1[:], zT[p2][:, i * NP:(i + 1) * NP],
                                     wo_sb[:, p2, 0:512],
                                     start=(p2 == 0), stop=(p2 == NPAIR - 1))
                o_t = o_pool.tile([NP, D], F32, tag="o", name=f"ot{i}")
                nc.vector.tensor_copy(o_t[:, 0:512], ps1[:])
                return o_t

            def emit_outproj_b(i, o_t):
                ps2 = psQK.tile([NP, 256], F32, tag="qk", name=f"op{i}b")
                for p2 in range(NPAIR):
                    nc.tensor.matmul(ps2[:], zT[p2][:, i * NP:(i + 1) * NP],
                                     wo_sb[:, p2, 512:768],
                                     start=(p2 == 0), stop=(p2 == NPAIR - 1))
                nc.vector.tensor_copy(o_t[:, 512:768], ps2[:])
                nc.sync.dma_start(out_d.ap()[i * NP:(i + 1) * NP, :], o_t[:])

            # ---- qh1 sweep (q 512:1024, all key tiles) ----
            o3 = None
            for p in range(5):
                pts = pass1(p, 512, 1024)
                if p < 3:
                    emit_outproj(p)
                elif p == 3:
                    o3 = emit_outproj_a(3)
                elif p == 4:
                    emit_outproj_b(3, o3)
                pass2(p, 512, 1024, pts)

            # final pair: two 256-wide chunks so outproj i4/i5 overlap chunk B
            pts = pass1(5, 512, 768)
            pass2(5, 512, 768, pts)
            pts = pass1(5, 768, 1024)
            emit_outproj(4)
            emit_outproj(5)
            pass2(5, 768, 1024, pts)
            emit_outproj(6, split_store=True)
            emit_outproj(7, split_store=True)


_NC = None


def _get_nc():
    global _NC
    if _NC is None:
        nc = _build()
        nc.m = get_hw_module(nc.m)
        _NC = nc
    return _NC


def _pack_wkq(w):
    # [H, D, Dh] -> [NPAIR, 128(d'), DT, 128(pair-col)] bf16
    import ml_dtypes
    w = np.asarray(w, dtype=np.float32)
    wt = w.transpose(1, 0, 2).reshape(DT, NP, NPAIR, NP)  # [dt, d', p, c]
    return np.ascontiguousarray(wt.transpose(2, 1, 0, 3)).astype(ml_dtypes.bfloat16)


def _in_maps(inputs):
    import ml_dtypes
    x = np.asarray(inputs["normalized_resid_pre"], dtype=np.float32)
    wv = np.asarray(inputs["W_V"], dtype=np.float32)
    wo = np.asarray(inputs["W_O"], dtype=np.float32)
    shared = {
        "wq": _pack_wkq(inputs["W_Q"]),
        "wk": _pack_wkq(inputs["W_K"]),
        "wv": np.ascontiguousarray(
            wv.transpose(1, 0, 2).reshape(DT, NP, H * Dh)
        ).astype(ml_dtypes.bfloat16),
        "wo": np.ascontiguousarray(wo.reshape(NPAIR, NP, D)).astype(ml_dtypes.bfloat16),
    }
    maps = []
    for b in range(B):
        xt = np.ascontiguousarray(x[b].T.reshape(DT, NP, S)).astype(ml_dtypes.bfloat16)
        maps.append(dict(
            shared,
            xa=np.ascontiguousarray(xt[:, :, 0:512]),
            xb=np.ascontiguousarray(xt[:, :, 512:1024]),
        ))
    return maps


def kernel(**inputs):
    nc = _get_nc()
    res = bass_utils.run_bass_kernel_spmd(nc, _in_maps(inputs), core_ids=list(range(B)))
    return np.stack([res.results[b]["out"] for b in range(B)], axis=0)


def kernel_traced(**inputs):
    """Like kernel() but also captures an NTFF profile (requires the ntff shim
    to be installed by the caller). Returns (out, BassKernelResults)."""
    nc = _get_nc()
    res = bass_utils.run_bass_kernel_spmd(
        nc, _in_maps(inputs), core_ids=list(range(B)), trace=True
    )
    out = np.stack([res.results[b]["out"] for b in range(B)], axis=0)
    return out, res


# revision 20
# speedup vs baseline: 1.0085x; 1.0085x over previous
"""Multi-head causal attention (B=8,S=1024,D=768,H=12,Dh=64) on 8 TRN2 NeuronCores.

Data-parallel over batch: each core handles one batch element end-to-end
(QKV projection -> causal softmax attention -> output projection). No
collectives. All matmuls run in bf16 (fp32 PSUM accumulation); inputs are
cast/packed to bf16 on the host.

Schedule (vs original baseline):
  - DMA staging: x as six 2KB-line tiles + per-pair weight chunks, split
    between the two HW DGE queues (sync/scalar) in consumption-priority
    order, later weights on GpSimd's SW DGE; the PE starts at ~12us instead
    of stalling to ~30us. Zero-filled biases are never loaded.
  - PE warmup burst while DMAs stream (HAM clock-gate releases at ~3.4us of
    sustained activity; real matmuls then run at 2.4GHz from the start).
  - qh-major attention sweep with fillers distributed so the PE never idles:
    qh0 sweep carries QK(p2..p5) + V(kt4..7), qh1 sweep carries the output
    projection for s<512 (i3 split across two groups); the final pair's qh1
    is split in two 256-wide chunks so outproj(i4,i5) overlaps its second
    half, and the last stores ship column-split (short tail).
  - Diagonal-block causal masks merged: one DVE multiply per block over both
    heads ([128,2,128] against a doubled triangular constant).
  - PSUM->SBUF copies spread: QK copies on ACT (idle during the qh0 sweep),
    V/z/out on DVE. No bias loads/adds (spec fills biases with zeros).

Per-core layout:
  xT[dt]    [128(d), 1024(s)] bf16  x^T, 6 d-tiles
  qT/kT[p]  [128(he-pair), 1024(s)] bf16  head-pair packed: partitions 0-63 =
            head 2p, partitions 64-127 = head 2p+1 (e fastest)
  v_sb[kt]  [128(s), 768(h e)] bf16  V in natural layout per key-tile
  Scores are kept transposed (S^T[k, q]) so the softmax reduction over k is
  a ones-matmul on PE; exp needs no max-subtraction (|s/8| < ~4 here).
  zT[p]     [128(he-pair), 1024(s)] bf16  normalized attention output
"""
import sys

sys.path.insert(0, "/opt/trn_rl_repo")

import numpy as np

import concourse.bacc as bacc
import concourse.mybir as mybir
from concourse import tile
from concourse import bass_utils
from concourse.bass_interp import get_hw_module

from concourse.masks import make_upper_triangular

F32 = mybir.dt.float32
BF16 = mybir.dt.bfloat16
EXP = mybir.ActivationFunctionType.Exp

B, S, D, H, Dh = 8, 1024, 768, 12, 64
NP = 128          # partitions
DT = D // NP      # 6 d-tiles
ST = S // NP      # 8 s-tiles
KT = S // NP      # 8 k-tiles
NPAIR = H // 2    # 6 head pairs
SCALE = 1.0 / 8.0  # 1/sqrt(Dh)


def _build():
    nc = bacc.Bacc(
        "TRN2",
        target_bir_lowering=False,
        debug=False,
        enable_asserts=False,
        num_devices=8,
    )
    xt_d = nc.dram_tensor("xt", (DT, NP, S), BF16, kind="ExternalInput")
    wq_d = nc.dram_tensor("wq", (NPAIR, NP, DT, NP), BF16, kind="ExternalInput")
    wk_d = nc.dram_tensor("wk", (NPAIR, NP, DT, NP), BF16, kind="ExternalInput")
    wv_d = nc.dram_tensor("wv", (DT, NP, H * Dh), BF16, kind="ExternalInput")
    wo_d = nc.dram_tensor("wo", (NPAIR, NP, D), BF16, kind="ExternalInput")
    out_d = nc.dram_tensor("out", (S, D), F32, kind="ExternalOutput")

    with tile.TileContext(nc) as tc:
        _body(tc, xt_d, wq_d, wk_d, wv_d, wo_d, out_d)

    nc.compile()
    return nc


def _body(tc, xt_d, wq_d, wk_d, wv_d, wo_d, out_d):
    nc = tc.nc

    with (
        tc.tile_pool(name="const", bufs=1) as const_pool,
        tc.tile_pool(name="qkT", bufs=1) as qkT_pool,
        tc.tile_pool(name="vsb", bufs=1) as v_pool,
        tc.tile_pool(name="zT", bufs=1) as zT_pool,
        tc.tile_pool(name="wsb", bufs=1) as w_pool,
        tc.tile_pool(name="xsb", bufs=1) as x_pool,
        tc.tile_pool(name="pt", bufs=12) as pt_pool,
        tc.tile_pool(name="rcp", bufs=2) as r_pool,
        tc.tile_pool(name="osb", bufs=3) as o_pool,
    ):
        # ---- constants (gpsimd; warm first so PE warmup starts ASAP) ----
        warm = const_pool.tile([NP, 512], BF16, tag="warm")
        nc.gpsimd.memset(warm[:], 0.125)
        ones64 = const_pool.tile([NP, 64], BF16, tag="ones64")
        nc.gpsimd.memset(ones64[:], 1.0)
        tri2 = const_pool.tile([NP, 2, NP], BF16, tag="tri2")
        make_upper_triangular(nc, tri2[:, 0, :], val=1.0, diag=True)
        make_upper_triangular(nc, tri2[:, 1, :], val=1.0, diag=True)

        # ---- persistent tiles ----
        qT = [qkT_pool.tile([NP, S], BF16, tag=f"qT{p}", name=f"qT{p}") for p in range(NPAIR)]
        kT = [qkT_pool.tile([NP, S], BF16, tag=f"kT{p}", name=f"kT{p}") for p in range(NPAIR)]
        v_sb = [v_pool.tile([NP, H * Dh], BF16, tag=f"v{k}", name=f"v{k}") for k in range(KT)]
        zT = [zT_pool.tile([NP, S], BF16, tag=f"zT{p}", name=f"zT{p}") for p in range(NPAIR)]
        xT = [x_pool.tile([NP, S], BF16, tag=f"xT{dt}", name=f"xT{dt}") for dt in range(DT)]
        wk_sb = [w_pool.tile([NP, DT, NP], BF16, tag=f"wk{p}", name=f"wk{p}") for p in range(NPAIR)]
        wq_sb = [w_pool.tile([NP, DT, NP], BF16, tag=f"wq{p}", name=f"wq{p}") for p in range(NPAIR)]
        wv_sb = w_pool.tile([NP, DT, H * Dh], BF16, tag="wv", name="wv_sb")
        wo_sb = w_pool.tile([NP, NPAIR, D], BF16, tag="wo", name="wo_sb")

        # ---- loads ----
        # x + first-pair weights on the two HW DGE queues (sync/scalar); all
        # later weights on GpSimd's SW DGE so semaphore-reuse waits on the
        # dma_start instructions never block an engine with compute queued
        # behind them (ACT does the QK copies, Sync stores the output).
        nc.sync.dma_start(wk_sb[0][:], wk_d.ap()[0])
        nc.sync.dma_start(wq_sb[0][:], wq_d.ap()[0])
        nc.sync.dma_start(xT[0][:], xt_d.ap()[0])
        nc.sync.dma_start(xT[2][:], xt_d.ap()[2])
        nc.sync.dma_start(xT[4][:], xt_d.ap()[4])

        nc.scalar.dma_start(xT[1][:], xt_d.ap()[1])
        nc.scalar.dma_start(xT[3][:], xt_d.ap()[3])
        nc.scalar.dma_start(xT[5][:], xt_d.ap()[5])

        nc.gpsimd.dma_start(wk_sb[1][:], wk_d.ap()[1])
        nc.gpsimd.dma_start(wq_sb[1][:], wq_d.ap()[1])
        nc.gpsimd.dma_start(wk_sb[2][:], wk_d.ap()[2])
        nc.gpsimd.dma_start(wq_sb[2][:], wq_d.ap()[2])
        nc.gpsimd.dma_start(wk_sb[3][:], wk_d.ap()[3])
        nc.gpsimd.dma_start(wq_sb[3][:], wq_d.ap()[3])
        nc.gpsimd.dma_start(wv_sb[:], wv_d.ap().rearrange("dt p c -> p dt c"))
        nc.gpsimd.dma_start(wk_sb[4][:], wk_d.ap()[4])
        nc.gpsimd.dma_start(wq_sb[4][:], wq_d.ap()[4])
        nc.gpsimd.dma_start(wk_sb[5][:], wk_d.ap()[5])
        nc.gpsimd.dma_start(wq_sb[5][:], wq_d.ap()[5])
        nc.gpsimd.dma_start(wo_sb[:], wo_d.ap().rearrange("q p c -> p q c"))

        with (
            tc.tile_pool(name="psQK", bufs=2, space="PSUM") as psQK,
            tc.tile_pool(name="psS", bufs=2, space="PSUM") as psS,
            tc.tile_pool(name="psZ", bufs=1, space="PSUM") as psZ,
            tc.tile_pool(name="psL", bufs=1, space="PSUM") as psL,
        ):
            # ---- PE warmup: release the HAM clock gate while DMAs stream ----
            for i in range(9):
                wps = psQK.tile([NP, 512], F32, tag="qk", name=f"warm{i}")
                nc.tensor.matmul(wps[:], warm[:, 0:128], warm[:], start=True, stop=True)

            def emit_qk_sc(p, which, sc):
                """One s-half of the K or Q projection for pair p."""
                w_sb, dstT = (wk_sb, kT) if which == 0 else (wq_sb, qT)
                ps = psQK.tile([NP, 512], F32, tag="qk", name=f"qk{p}_{which}_{sc}")
                for dt in range(DT):
                    nc.tensor.matmul(
                        ps[:], w_sb[p][:, dt, :], xT[dt][:, sc * 512:(sc + 1) * 512],
                        start=(dt == 0), stop=(dt == DT - 1),
                    )
                nc.scalar.copy(dstT[p][:, sc * 512:(sc + 1) * 512], ps[:])

            def emit_qk(p):
                emit_qk_sc(p, 0, 0)
                emit_qk_sc(p, 1, 0)
                emit_qk_sc(p, 0, 1)
                emit_qk_sc(p, 1, 1)

            def emit_v(kt):
                """V projection for one key tile (all 12 heads)."""
                ps = psS.tile([NP, 1024], F32, tag="st", name=f"vps{kt}")
                for dt in range(DT):
                    lhs = xT[dt][:, kt * NP:(kt + 1) * NP]
                    nc.tensor.matmul(ps[:, 0:512], lhs, wv_sb[:, dt, 0:512],
                                     start=(dt == 0), stop=(dt == DT - 1))
                    nc.tensor.matmul(ps[:, 512:768], lhs, wv_sb[:, dt, 512:768],
                                     start=(dt == 0), stop=(dt == DT - 1))
                nc.vector.tensor_copy(v_sb[kt][:], ps[:, 0:768])

            def emit_outproj(i, split_store=False):
                ps1 = psQK.tile([NP, 512], F32, tag="qk", name=f"op{i}a")
                ps2 = psQK.tile([NP, 256], F32, tag="qk", name=f"op{i}b")
                for p2 in range(NPAIR):
                    lhs = zT[p2][:, i * NP:(i + 1) * NP]
                    nc.tensor.matmul(ps1[:], lhs, wo_sb[:, p2, 0:512],
                                     start=(p2 == 0), stop=(p2 == NPAIR - 1))
                for p2 in range(NPAIR):
                    lhs = zT[p2][:, i * NP:(i + 1) * NP]
                    nc.tensor.matmul(ps2[:], lhs, wo_sb[:, p2, 512:768],
                                     start=(p2 == 0), stop=(p2 == NPAIR - 1))
                o_t = o_pool.tile([NP, D], F32, tag="o", name=f"ot{i}")
                if split_store:
                    # tail tiles: copies on ACT (idle after the last exp) and
                    # ship the first 2/3 while ps2 still accumulates
                    nc.scalar.copy(o_t[:, 0:512], ps1[:])
                    nc.sync.dma_start(out_d.ap()[i * NP:(i + 1) * NP, 0:512], o_t[:, 0:512])
                    nc.scalar.copy(o_t[:, 512:768], ps2[:])
                    nc.sync.dma_start(out_d.ap()[i * NP:(i + 1) * NP, 512:768], o_t[:, 512:768])
                else:
                    nc.vector.tensor_copy(o_t[:, 0:512], ps1[:])
                    nc.vector.tensor_copy(o_t[:, 512:768], ps2[:])
                    nc.sync.dma_start(out_d.ap()[i * NP:(i + 1) * NP, :], o_t[:])

            def blocks(qlo, qhi):
                """(kt, c0, w) for every score block intersecting q in [qlo,qhi)."""
                out = []
                for kt in range(KT):
                    c0 = max(kt * NP, qlo)
                    if c0 < qhi:
                        out.append((kt, c0, qhi - c0))
                return out

            def pass1(p, qlo, qhi):
                """Scores + exp (+ causal mask on diagonal blocks)."""
                pts = {}
                for kt, c0, w in blocks(qlo, qhi):
                    q0 = kt * NP
                    st = psS.tile([NP, 2, 512], F32, tag="st", name=f"st{p}_{qlo}_{kt}")
                    for h in range(2):
                        nc.tensor.matmul(
                            st[:, h, 0:w],
                            kT[p][h * 64:(h + 1) * 64, q0:q0 + NP],
                            qT[p][h * 64:(h + 1) * 64, c0:c0 + w],
                            start=True, stop=True,
                        )
                    pt = pt_pool.tile([NP, 2, 512], BF16, tag="pt", name=f"pt{p}_{qlo}_{kt}")
                    nc.scalar.activation(pt[:, :, 0:w], st[:, :, 0:w], EXP, scale=SCALE)
                    if c0 == q0:  # diagonal block: zero out k > q (both heads at once)
                        nc.vector.tensor_mul(pt[:, :, 0:NP], pt[:, :, 0:NP], tri2[:])
                    pts[kt] = (pt, c0, w)
                return pts

            def pass2(p, qlo, qhi, pts):
                """l and z accumulation + softmax normalize into zT."""
                width = qhi - qlo
                bl = blocks(qlo, qhi)
                z_ps = psZ.tile([NP, width], F32, tag="z", name=f"z{p}_{qlo}")
                l_ps = psL.tile([NP, width], F32, tag="l", name=f"l{p}_{qlo}")
                for idx, (kt, c0, w) in enumerate(bl):
                    pt, _, _ = pts[kt]
                    first = idx == 0
                    last = idx == len(bl) - 1
                    o0 = c0 - qlo

                    # pair l(h) with z(1-h): disjoint PE col groups + PSUM banks
                    def mm_l(h):
                        nc.tensor.matmul(
                            l_ps[h * 64:(h + 1) * 64, o0:o0 + w],
                            ones64[:, 0:64], pt[:, h, 0:w],
                            start=first, stop=last, skip_group_check=True,
                        )

                    def mm_z(h):
                        nc.tensor.matmul(
                            z_ps[h * 64:(h + 1) * 64, o0:o0 + w],
                            v_sb[kt][:, (2 * p + h) * 64:(2 * p + h + 1) * 64],
                            pt[:, h, 0:w],
                            start=first, stop=last, skip_group_check=True,
                        )

                    mm_l(0); mm_z(1); mm_l(1); mm_z(0)
                recip = r_pool.tile([NP, 512], F32, tag="rcp", name=f"rcp{p}_{qlo}")
                nc.vector.reciprocal_approx_fast(out=recip[:, 0:width], in_=l_ps[:])
                nc.vector.tensor_mul(zT[p][:, qlo:qhi], z_ps[:], recip[:, 0:width])

            # ---- projections head start ----
            emit_qk(0)
            emit_qk(1)
            for kt in range(4):
                emit_v(kt)

            # ---- qh0 sweep (q 0:512, key tiles 0..3) with fillers ----
            qh0_fill = {0: lambda: emit_qk(2), 1: lambda: emit_qk(3),
                        2: lambda: emit_qk(4), 3: lambda: emit_qk(5),
                        4: lambda: (emit_v(4), emit_v(5)),
                        5: lambda: (emit_v(6), emit_v(7))}
            for p in range(NPAIR):
                pts = pass1(p, 0, 512)
                qh0_fill[p]()
                pass2(p, 0, 512, pts)

            def emit_outproj_a(i):
                ps1 = psQK.tile([NP, 512], F32, tag="qk", name=f"op{i}a")
                for p2 in range(NPAIR):
                    nc.tensor.matmul(ps1[:], zT[p2][:, i * NP:(i + 1) * NP],
                                     wo_sb[:, p2, 0:512],
                                     start=(p2 == 0), stop=(p2 == NPAIR - 1))
                o_t = o_pool.tile([NP, D], F32, tag="o", name=f"ot{i}")
                nc.vector.tensor_copy(o_t[:, 0:512], ps1[:])
                return o_t

            def emit_outproj_b(i, o_t):
                ps2 = psQK.tile([NP, 256], F32, tag="qk", name=f"op{i}b")
                for p2 in range(NPAIR):
                    nc.tensor.matmul(ps2[:], zT[p2][:, i * NP:(i + 1) * NP],
                                     wo_sb[:, p2, 512:768],
                                     start=(p2 == 0), stop=(p2 == NPAIR - 1))
                nc.vector.tensor_copy(o_t[:, 512:768], ps2[:])
                nc.sync.dma_start(out_d.ap()[i * NP:(i + 1) * NP, :], o_t[:])

            # ---- qh1 sweep (q 512:1024, all key tiles) ----
            o3 = None
            for p in range(5):
                pts = pass1(p, 512, 1024)
                if p < 3:
                    emit_outproj(p)
                elif p == 3:
                    o3 = emit_outproj_a(3)
                elif p == 4:
                    emit_outproj_b(3, o3)
                pass2(p, 512, 1024, pts)

            # final pair: two 256-wide chunks so outproj i4/i5 overlap chunk B
            pts = pass1(5, 512, 768)
            pass2(5, 512, 768, pts)
            pts = pass1(5, 768, 1024)
            emit_outproj(4)
            emit_outproj(5)
            pass2(5, 768, 1024, pts)
            emit_outproj(6, split_store=True)
            emit_outproj(7, split_store=True)


_NC = None


def _get_nc():
    global _NC
    if _NC is None:
        nc = _build()
        nc.m = get_hw_module(nc.m)
        _NC = nc
    return _NC


def _pack_wkq(w):
    # [H, D, Dh] -> [NPAIR, 128(d'), DT, 128(pair-col)] bf16
    import ml_dtypes
    w = np.asarray(w, dtype=np.float32)
    wt = w.transpose(1, 0, 2).reshape(DT, NP, NPAIR, NP)  # [dt, d', p, c]
    return np.ascontiguousarray(wt.transpose(2, 1, 0, 3)).astype(ml_dtypes.bfloat16)


def _in_maps(inputs):
    import ml_dtypes
    x = np.asarray(inputs["normalized_resid_pre"], dtype=np.float32)
    wv = np.asarray(inputs["W_V"], dtype=np.float32)
    wo = np.asarray(inputs["W_O"], dtype=np.float32)
    shared = {
        "wq": _pack_wkq(inputs["W_Q"]),
        "wk": _pack_wkq(inputs["W_K"]),
        "wv": np.ascontiguousarray(
            wv.transpose(1, 0, 2).reshape(DT, NP, H * Dh)
        ).astype(ml_dtypes.bfloat16),
        "wo": np.ascontiguousarray(wo.reshape(NPAIR, NP, D)).astype(ml_dtypes.bfloat16),
    }
    maps = []
    for b in range(B):
        maps.append(dict(
            shared,
            xt=np.ascontiguousarray(x[b].T.reshape(DT, NP, S)).astype(ml_dtypes.bfloat16),
        ))
    return maps


def kernel(**inputs):
    nc = _get_nc()
    res = bass_utils.run_bass_kernel_spmd(nc, _in_maps(inputs), core_ids=list(range(B)))
    return np.stack([res.results[b]["out"] for b in range(B)], axis=0)


def kernel_traced(**inputs):
    """Like kernel() but also captures an NTFF profile (requires the ntff shim
    to be installed by the caller). Returns (out, BassKernelResults)."""
    nc = _get_nc()
    res = bass_utils.run_bass_kernel_spmd(
        nc, _in_maps(inputs), core_ids=list(range(B)), trace=True
    )
    out = np.stack([res.results[b]["out"] for b in range(B)], axis=0)
    return out, res
